# revision 5
# baseline (speedup 1.0000x reference)
"""Differential attention (B=2, S=2048, HS=1024, H=16, KV=4, D=64) on 8 trn2 cores.

Sharding: core c = (b, g) with b = c // 4 (data parallel on batch) and
g = c % 4 (tensor parallel over the 4 KV head groups; each core owns the
4 query heads of its group).  Each core computes its 4 heads' normed
attention output and a row-parallel partial of the output projection
(out_pt = (O_heads @ Wo_rows)^T); the host sums the 4 partials per batch.

Device pipeline per core (all matmuls fp32r ~= tf32):
  xT[1024,2048] -> Q^T/K^T/V^T projections (PE) with RoPE fused
  (partition-swap via SBUF->SBUF DMA, sign baked into the sin table);
  flash-style causal attention per (head, q-tile of 512):
    S^T[k,q] = K^T-strip.T @ Q^T-strip  (two 64-contraction row-strips),
    causal diag handled by a bf16 identity@mask accumulate-matmul,
    P = exp(S/8) on ACT (no row-max: scores are O(5), fp32 exp is safe),
    U^T[128,q] += [V|ones].T @ P  (ones block replicates the softmax
    denominator onto partitions 64..127),
  epilogue: lane-aligned reciprocal of the denominator rows, SBUF->SBUF
  partition shift, O = U1/r1 - lam*U2/r2,
  RMSNorm via ones-matmul row-sums of O^2 + ln/exp rsqrt (same ACT table),
  subln_w folded into Wo rows on the host.
"""

import math
import sys

import numpy as np

try:
    import concourse.bass as bass  # noqa: F401
except ImportError:
    sys.path.insert(0, "/opt/trn_rl_repo")

import concourse.bass as bass
import concourse.tile as tile
from concourse import bacc, mybir
from concourse import bass_utils

f32 = mybir.dt.float32
f32r = mybir.dt.float32r
bf16 = mybir.dt.bfloat16
AF = mybir.ActivationFunctionType
ALU = mybir.AluOpType

B, S, HS = 2, 2048, 1024
H, KV, D = 2, 4, 64  # H patched below; keep line for clarity
H = 16
NHL = 4            # query heads per core
NQT = 4            # q tiles of 512
QTW = 512
NKT = 16           # k tiles of 128
NHS = 8            # hs tiles of 128
NEG = -1e9
EPS = 1e-5

_prog_cache = {}


def _build_program(lam: float):
    nc = bacc.Bacc("TRN2", target_bir_lowering=False, debug=False,
                   enable_asserts=False, num_devices=8)

    xt = nc.dram_tensor("xt", [HS, S], f32r, kind="ExternalInput").ap()
    wq = nc.dram_tensor("wq", [HS, 512], f32r, kind="ExternalInput").ap()
    wk = nc.dram_tensor("wk", [HS, 128], f32r, kind="ExternalInput").ap()
    wv = nc.dram_tensor("wv", [HS, 64], f32r, kind="ExternalInput").ap()
    wo = nc.dram_tensor("wo", [256, HS], f32r, kind="ExternalInput").ap()
    cos_t = nc.dram_tensor("cos_t", [128, S], f32, kind="ExternalInput").ap()
    sin_t = nc.dram_tensor("sin_t", [128, S], f32, kind="ExternalInput").ap()
    identb = nc.dram_tensor("identb", [128, 128], bf16, kind="ExternalInput").ap()
    maskb = nc.dram_tensor("maskb", [128, 128], bf16, kind="ExternalInput").ap()
    idf = nc.dram_tensor("idf", [64, 64], f32, kind="ExternalInput").ap()
    ones = nc.dram_tensor("ones", [128, 64], f32r, kind="ExternalInput").ap()
    out_pt = nc.dram_tensor("out_pt", [HS, S], f32, kind="ExternalOutput").ap()

    with tile.TileContext(nc) as tc:
        with tc.tile_pool(name="persist", bufs=1) as pp:
            identb_sb = pp.tile([128, 128], bf16, name="identb", tag="identb")
            nc.sync.dma_start(identb_sb[:], identb[:])
            maskb_sb = pp.tile([128, 128], bf16, name="maskb", tag="maskb")
            nc.sync.dma_start(maskb_sb[:], maskb[:])
            ones_sb = pp.tile([128, 64], f32r, name="ones", tag="ones")
            nc.sync.dma_start(ones_sb[:], ones[:])
            wo_sb = []
            for t in range(2):
                w = pp.tile([128, HS], f32r, name=f"wo{t}", tag=f"wo{t}")
                nc.sync.dma_start(w[:], wo[t * 128:(t + 1) * 128, :])
                wo_sb.append(w)
            q_sb = [pp.tile([128, S], f32r, name=f"q{j}", tag=f"q{j}")
                    for j in range(NHL)]
            k_sb = pp.tile([128, S], f32r, name="k", tag="k")
            va = [pp.tile([128, 128], f32r, name=f"va{kt}", tag=f"va{kt}")
                  for kt in range(NKT)]
            vb = [pp.tile([128, 128], f32r, name=f"vb{kt}", tag=f"vb{kt}")
                  for kt in range(NKT)]
            o_sb = [pp.tile([128, S], f32, name=f"o{t}", tag=f"o{t}")
                    for t in range(2)]

            # ---------------- projections + RoPE ----------------
            with tc.tile_pool(name="proj", bufs=1) as pj, \
                 tc.tile_pool(name="pwork", bufs=2) as pw, \
                 tc.psum_pool(name="ppj", bufs=2) as ppj:
                wq_sb = []
                for hs in range(NHS):
                    t_ = pj.tile([128, 512], f32r, name=f"wq{hs}", tag=f"wq{hs}")
                    nc.sync.dma_start(t_[:], wq[hs * 128:(hs + 1) * 128, :])
                    wq_sb.append(t_)
                wk_sb = []
                for hs in range(NHS):
                    t_ = pj.tile([128, 128], f32r, name=f"wk{hs}", tag=f"wk{hs}")
                    nc.sync.dma_start(t_[:], wk[hs * 128:(hs + 1) * 128, :])
                    wk_sb.append(t_)
                wv_sb = []
                for hs in range(NHS):
                    t_ = pj.tile([128, 64], f32r, name=f"wv{hs}", tag=f"wv{hs}")
                    nc.sync.dma_start(t_[:], wv[hs * 128:(hs + 1) * 128, :])
                    wv_sb.append(t_)
                cos_sb = pj.tile([128, S], f32, name="cos", tag="cos")
                nc.sync.dma_start(cos_sb[:], cos_t[:])
                sin_sb = pj.tile([128, S], f32, name="sin", tag="sin")
                nc.sync.dma_start(sin_sb[:], sin_t[:])
                idf_sb = pj.tile([64, 64], f32, name="idf", tag="idf")
                nc.sync.dma_start(idf_sb[:], idf[:])
                vt_sb = pj.tile([64, S], f32, name="vt", tag="vt")

                def rope_block(ps, dst, qlo, qhi):
                    # dst = ps * cos + swap32(ps) * sin   (sign baked in sin)
                    qpl = pw.tile([128, QTW], f32, name="qpl", tag="qpl")
                    nc.scalar.copy(qpl[:], ps[:])
                    qsw = pw.tile([128, QTW], f32, name="qsw", tag="qsw")
                    for blk, src in ((0, 32), (1, 0), (2, 96), (3, 64)):
                        nc.sync.dma_start(qsw[blk * 32:(blk + 1) * 32, :],
                                          qpl[src:src + 32, :])
                    qc = pw.tile([128, QTW], f32, name="qc", tag="qc")
                    nc.vector.tensor_mul(qc[:], qpl[:], cos_sb[:, qlo:qhi])
                    qs = pw.tile([128, QTW], f32, name="qs", tag="qs")
                    nc.vector.tensor_mul(qs[:], qsw[:], sin_sb[:, qlo:qhi])
                    nc.vector.tensor_add(dst, qc[:], qs[:])

                for qt in range(NQT):
                    qlo, qhi = qt * QTW, (qt + 1) * QTW
                    xt_sb = []
                    for hs in range(NHS):
                        t_ = pw.tile([128, QTW], f32r, name=f"xt{hs}",
                                     tag=f"xt{hs}")
                        nc.sync.dma_start(t_[:], xt[hs * 128:(hs + 1) * 128,
                                                    qlo:qhi])
                        xt_sb.append(t_)
                    for j in range(NHL):
                        psq = ppj.tile([128, QTW], f32, name="psQ", tag="psQ")
                        for hs in range(NHS):
                            nc.tensor.matmul(
                                psq[:], wq_sb[hs][:, j * 128:(j + 1) * 128],
                                xt_sb[hs][:],
                                start=(hs == 0), stop=(hs == NHS - 1))
                        rope_block(psq, q_sb[j][:, qlo:qhi], qlo, qhi)
                    psk = ppj.tile([128, QTW], f32, name="psQ", tag="psQ")
                    for hs in range(NHS):
                        nc.tensor.matmul(psk[:], wk_sb[hs][:], xt_sb[hs][:],
                                         start=(hs == 0), stop=(hs == NHS - 1))
                    rope_block(psk, k_sb[:, qlo:qhi], qlo, qhi)
                    psv = ppj.tile([64, QTW], f32, name="psV", tag="psV")
                    for hs in range(NHS):
                        nc.tensor.matmul(psv[:], wv_sb[hs][:], xt_sb[hs][:],
                                         start=(hs == 0), stop=(hs == NHS - 1))
                    nc.scalar.copy(vt_sb[:, qlo:qhi], psv[:])

                # V^T -> V tiles [128k, 64] (+ ones block; vb is lam-scaled)
                for kt in range(NKT):
                    psvt = ppj.tile([128, 64], f32, name="psVT", tag="psVT")
                    nc.tensor.transpose(psvt[:], vt_sb[:, kt * 128:(kt + 1) * 128],
                                        idf_sb[:])
                    nc.scalar.copy(va[kt][:, 0:64], psvt[:])
                    nc.scalar.mul(vb[kt][:, 0:64], psvt[:], lam)
                    nc.vector.tensor_copy(va[kt][:, 64:128], ones_sb[:])
                    nc.vector.tensor_copy(vb[kt][:, 64:128], ones_sb[:])

            # ---------------- attention ----------------
            with tc.tile_pool(name="att", bufs=3) as pa, \
                 tc.tile_pool(name="ep", bufs=2) as pe, \
                 tc.psum_pool(name="pat", bufs=2) as ppa:
                for j in range(NHL):
                    for qt in range(NQT):
                        qbase = qt * QTW
                        last_kt = 4 * qt + 3
                        psu1 = ppa.tile([128, QTW], f32, name="psU1", tag="psU1")
                        psu2 = ppa.tile([128, QTW], f32, name="psU2", tag="psU2")
                        for kt in range(last_kt + 1):
                            jd = kt - 4 * qt
                            q0 = 128 * jd if jd >= 0 else 0
                            qlo, qhi = qbase + q0, qbase + QTW
                            diag = jd >= 0
                            pss = ppa.tile([128, 2 * QTW], f32, name="psS", tag="psS")
                            nc.tensor.matmul(
                                pss[:, q0:QTW],
                                k_sb[0:64, kt * 128:(kt + 1) * 128],
                                q_sb[j][0:64, qlo:qhi],
                                start=True, stop=not diag, skip_group_check=True)
                            nc.tensor.matmul(
                                pss[:, QTW + q0:2 * QTW],
                                k_sb[64:128, kt * 128:(kt + 1) * 128],
                                q_sb[j][64:128, qlo:qhi],
                                start=True, stop=not diag, skip_group_check=True)
                            if diag:
                                nc.tensor.matmul(
                                    pss[:, q0:q0 + 128], identb_sb[:], maskb_sb[:],
                                    start=False, stop=True, skip_group_check=True)
                                nc.tensor.matmul(
                                    pss[:, QTW + q0:QTW + q0 + 128],
                                    identb_sb[:], maskb_sb[:],
                                    start=False, stop=True, skip_group_check=True)
                            p12 = pa.tile([128, 2 * QTW], f32r, name="p12", tag="p12")
                            nc.scalar.activation(p12[:, q0:2 * QTW],
                                                 pss[:, q0:2 * QTW],
                                                 AF.Exp, scale=0.125)
                            nc.tensor.matmul(
                                psu1[:, q0:QTW], va[kt][:], p12[:, q0:QTW],
                                start=(kt == 0), stop=(kt == last_kt),
                                skip_group_check=True)
                            nc.tensor.matmul(
                                psu2[:, q0:QTW], vb[kt][:], p12[:, QTW + q0:2 * QTW],
                                start=(kt == 0), stop=(kt == last_kt),
                                skip_group_check=True)
                        # epilogue: O^T = U1/r1 - lam*U2/r2
                        w1i = pe.tile([128, QTW], f32, name="w1i", tag="w1i")
                        nc.vector.reciprocal(w1i[64:128, :], psu1[64:128, :])
                        nc.sync.dma_start(w1i[0:64, :], w1i[64:128, :])
                        w2i = pe.tile([128, QTW], f32, name="w2i", tag="w2i")
                        nc.vector.reciprocal(w2i[64:128, :], psu2[64:128, :])
                        nc.sync.dma_start(w2i[0:64, :], w2i[64:128, :])
                        t1 = pe.tile([64, QTW], f32, name="t1", tag="t1")
                        nc.vector.tensor_mul(t1[:], psu1[0:64, :], w1i[0:64, :])
                        t2 = pe.tile([64, QTW], f32, name="t2", tag="t2")
                        nc.vector.tensor_mul(t2[:], psu2[0:64, :], w2i[0:64, :])
                        if j % 2 == 0:
                            nc.vector.tensor_sub(
                                o_sb[j // 2][0:64, qbase:qbase + QTW], t1[:], t2[:])
                        else:
                            otmp = pe.tile([64, QTW], f32, name="otmp", tag="otmp")
                            nc.vector.tensor_sub(otmp[:], t1[:], t2[:])
                            nc.sync.dma_start(
                                o_sb[j // 2][64:128, qbase:qbase + QTW], otmp[:])

            # ---------------- RMSNorm + output projection ----------------
            with tc.tile_pool(name="rms", bufs=2) as pr, \
                 tc.psum_pool(name="prs", bufs=2) as pps:
                on_sb = [pr.tile([128, S], f32r, name=f"on{t}", tag=f"on{t}",
                                 bufs=1) for t in range(2)]
                eps_sb = pr.tile([1, 1], f32, name="eps", tag="eps", bufs=1)
                nc.vector.memset(eps_sb[:], EPS)
                for pt in range(2):
                    osq = pr.tile([128, S], f32r, name="osq", tag="osq")
                    nc.vector.tensor_mul(osq[:], o_sb[pt][:], o_sb[pt][:])
                    for half in range(2):
                        hlo = half * 64
                        for qt in range(NQT):
                            qlo, qhi = qt * QTW, (qt + 1) * QTW
                            psss = pps.tile([1, QTW], f32, name="psSS", tag="psSS")
                            nc.tensor.matmul(
                                psss[:], ones_sb[hlo:hlo + 64, 0:1],
                                osq[hlo:hlo + 64, qlo:qhi],
                                start=True, stop=True)
                            lnq = pr.tile([1, QTW], f32, name="lnq", tag="lnq")
                            nc.scalar.activation(lnq[:], psss[:], AF.Ln,
                                                 scale=1.0 / 64.0,
                                                 bias=eps_sb[0:1, 0:1])
                            rmq = pr.tile([1, QTW], f32, name="rmq", tag="rmq")
                            nc.scalar.activation(rmq[:], lnq[:], AF.Exp, scale=-0.5)
                            rsb = pr.tile([128, QTW], f32, name="rsb", tag="rsb")
                            nc.gpsimd.partition_broadcast(rsb[:], rmq[0:1, :])
                            nc.vector.tensor_mul(
                                on_sb[pt][hlo:hlo + 64, qlo:qhi],
                                o_sb[pt][hlo:hlo + 64, qlo:qhi],
                                rsb[hlo:hlo + 64, :])

                for oc in range(8):
                    for qt in range(NQT):
                        qlo, qhi = qt * QTW, (qt + 1) * QTW
                        psw = pps.tile([128, QTW], f32, name="psW", tag="psW")
                        nc.tensor.matmul(psw[:], wo_sb[0][:, oc * 128:(oc + 1) * 128],
                                         on_sb[0][:, qlo:qhi], start=True, stop=False)
                        nc.tensor.matmul(psw[:], wo_sb[1][:, oc * 128:(oc + 1) * 128],
                                         on_sb[1][:, qlo:qhi], start=False, stop=True)
                        ow = pr.tile([128, QTW], f32, name="ow", tag="ow")
                        if (oc + qt) % 2 == 0:
                            nc.scalar.copy(ow[:], psw[:])
                        else:
                            nc.vector.tensor_copy(ow[:], psw[:])
                        nc.sync.dma_start(out_pt[oc * 128:(oc + 1) * 128, qlo:qhi],
                                          ow[:])

    nc.compile()
    return nc


def get_program(lam: float):
    key = round(float(lam), 9)
    if key not in _prog_cache:
        _prog_cache[key] = _build_program(float(lam))
    return _prog_cache[key]


def ml_bf16():
    import ml_dtypes
    return ml_dtypes.bfloat16


def _host_inputs(x, rope_cos, rope_sin, Wq, Wk, Wv, Wo, subln_w, lam):
    cos_t = np.ascontiguousarray(np.tile(rope_cos.T, (4, 1))).astype(np.float32)
    sin64 = np.concatenate([-rope_sin.T, rope_sin.T], axis=0)
    sin_t = np.ascontiguousarray(np.tile(sin64, (2, 1))).astype(np.float32)
    kk, qq = np.arange(128)[:, None], np.arange(128)[None, :]
    maskb = np.where(kk <= qq, 0.0, NEG).astype(ml_bf16())
    identb = np.eye(128).astype(ml_bf16())
    idf = np.eye(64, dtype=np.float32)
    ones = np.ones((128, 64), np.float32)
    sub4 = np.tile(subln_w.astype(np.float32), 4)[:, None]

    in_maps = []
    for c in range(8):
        b, g = c // 4, c % 4
        xt = np.ascontiguousarray(x[b].T).astype(np.float32)
        cols = []
        for j in range(NHL):
            h = 4 * g + j
            cols.append(Wq[:, h * 64:(h + 1) * 64])
            cols.append(Wq[:, (H + h) * 64:(H + h + 1) * 64])
        wq_c = np.ascontiguousarray(np.concatenate(cols, axis=1)).astype(np.float32)
        wk_c = np.ascontiguousarray(np.concatenate(
            [Wk[:, g * 64:(g + 1) * 64], Wk[:, (KV + g) * 64:(KV + g + 1) * 64]],
            axis=1)).astype(np.float32)
        wv_c = np.ascontiguousarray(Wv[:, g * 64:(g + 1) * 64]).astype(np.float32)
        wo_c = np.ascontiguousarray(
            Wo[g * 256:(g + 1) * 256, :] * sub4).astype(np.float32)
        in_maps.append({
            "xt": xt, "wq": wq_c, "wk": wk_c, "wv": wv_c, "wo": wo_c,
            "cos_t": cos_t, "sin_t": sin_t, "identb": identb, "maskb": maskb,
            "idf": idf, "ones": ones,
        })
    return in_maps


def _compute_lam(lambda_q1, lambda_k1, lambda_q2, lambda_k2):
    li = 0.8 - 0.6 * math.exp(-0.3)
    l1 = np.exp(np.dot(lambda_q1.astype(np.float32), lambda_k1.astype(np.float32)))
    l2 = np.exp(np.dot(lambda_q2.astype(np.float32), lambda_k2.astype(np.float32)))
    return float(l1 - l2 + li)


def _numpy_reference(x, rope_cos, rope_sin, attention_mask, Wq, Wk, Wv, Wo,
                     lambda_q1, lambda_k1, lambda_q2, lambda_k2, subln_w):
    """Pure-numpy fallback, only used if the mask is not the expected causal one."""
    bsz, seq_len, _ = x.shape

    def rope(t):
        c = np.concatenate([rope_cos, rope_cos], axis=-1)[None, None]
        s = np.concatenate([rope_sin, rope_sin], axis=-1)[None, None]
        t1, t2 = np.split(t, 2, axis=-1)
        rot = np.concatenate([-t2, t1], axis=-1)
        return t * c + rot * s

    q = (x @ Wq).reshape(bsz, seq_len, 2 * H, D)
    q1 = np.transpose(q[:, :, :H], (0, 2, 1, 3))
    q2 = np.transpose(q[:, :, H:], (0, 2, 1, 3))
    k = (x @ Wk).reshape(bsz, seq_len, 2 * KV, D)
    k1 = np.transpose(k[:, :, :KV], (0, 2, 1, 3))
    k2 = np.transpose(k[:, :, KV:], (0, 2, 1, 3))
    v = np.transpose((x @ Wv).reshape(bsz, seq_len, KV, D), (0, 2, 1, 3))
    q1, q2, k1, k2 = rope(q1), rope(q2), rope(k1), rope(k2)
    gr = H // KV
    k1 = np.repeat(k1, gr, axis=1)
    k2 = np.repeat(k2, gr, axis=1)
    v = np.repeat(v, gr, axis=1)
    scale = 1.0 / math.sqrt(D)

    def smax(a):
        a = a - a.max(axis=-1, keepdims=True)
        e = np.exp(a)
        return e / e.sum(axis=-1, keepdims=True)

    a1 = smax(np.einsum("bhqd,bhkd->bhqk", q1, k1) * scale + attention_mask)
    a2 = smax(np.einsum("bhqd,bhkd->bhqk", q2, k2) * scale + attention_mask)
    lam = _compute_lam(lambda_q1, lambda_k1, lambda_q2, lambda_k2)
    attn = a1 - lam * a2
    out = np.einsum("bhqk,bhkd->bhqd", attn, v)
    inv = 1.0 / np.sqrt(np.mean(out * out, axis=-1, keepdims=True) + EPS)
    out = out * inv * subln_w
    out = np.transpose(out, (0, 2, 1, 3)).reshape(bsz, seq_len, HS)
    return (out @ Wo).astype(np.float32)


LAST_RESULT = None


def kernel(x, rope_cos, rope_sin, attention_mask, Wq, Wk, Wv, Wo,
           lambda_q1, lambda_k1, lambda_q2, lambda_k2, subln_w):
    global LAST_RESULT
    x = np.asarray(x, np.float32)
    kk, qq = np.arange(S)[:, None], np.arange(S)[None, :]
    causal = np.where(qq <= kk, 0.0, NEG).astype(np.float32)[None, None]
    am = np.asarray(attention_mask, np.float32)
    if am.shape != (1, 1, S, S) or not np.array_equal(am, causal):
        return _numpy_reference(x, rope_cos, rope_sin, am, Wq, Wk, Wv, Wo,
                                lambda_q1, lambda_k1, lambda_q2, lambda_k2,
                                subln_w)

    lam = _compute_lam(lambda_q1, lambda_k1, lambda_q2, lambda_k2)
    nc = get_program(lam)
    in_maps = _host_inputs(x, np.asarray(rope_cos, np.float32),
                           np.asarray(rope_sin, np.float32),
                           np.asarray(Wq, np.float32), np.asarray(Wk, np.float32),
                           np.asarray(Wv, np.float32), np.asarray(Wo, np.float32),
                           np.asarray(subln_w, np.float32), lam)
    res = bass_utils.run_bass_kernel_spmd(nc, in_maps, core_ids=list(range(8)))
    LAST_RESULT = res
    y = np.zeros((B, S, HS), np.float32)
    for c in range(8):
        y[c // 4] += res.results[c]["out_pt"].T
    return y
